# revision 42
# baseline (speedup 1.0000x reference)
"""Trainium2 Bass kernel for nn_Aggregator (gnn_message_passing).

pooled[B,D] = owner_masks.f32 @ ((nodes@Wt + bt) * sigmoid(nodes@Wg + bg))

Sharding: nodes (and owner_masks columns) split along N across 8 cores;
the host sums the 8 partial results and applies the small bt correction
for "W-type" chunks (see below).

Design (cost-model exec ~115.9us vs 141.5us for the v1 kernel; engine
busy: PE ~105, DVE ~99, Pool ~94, DMA ~93, ACT ~78):
 - Per chunk (8 tiles of 128 nodes): 16 mm1 matmuls -> psum_d [128,1024]
   and psum_g split into two [128,512] half-tiles. Asymmetric PSUM pools
   (psum_d 2 bufs x 2 banks, psum_g halves 3 bufs x 1 bank, [B,2D]
   accumulator 1 bank = 8 banks) give 2 chunks of rotation lookahead;
   symmetric 3-slot pools stalled ~1us/chunk on the rotation edges.
 - The gates bias bg enters psum_g via rank-1 K=1 PE matmuls
   (ones[1,128].T @ bgb_row, 2x213ns) on ~80% of chunks; the rest use a
   DVE tensor_add into a gpre tile to balance PE vs DVE (the g-bias is
   the only work that can move between those two engines). ACT reads
   sigmoid straight from PSUM on rank-1 chunks, one op per half, so
   each psum_g half frees early.
 - The data bias bt rides free on the psum_d eviction: one DVE
   tensor_tensor add (psum fp32 + fp16 btb -> fp16 msg). W-type chunks
   (3 of 62) instead evict with an ACT copy (no bias) and run a
   256-wide mm2 over [msg|G], accumulating M@G into pool12[:, D:]; the
   host applies pooled += (M@G)_W * bt exactly. Chunk 0 is W-type and
   its first wide mm2 (start=True) initializes the whole accumulator.
 - msg and gates share a [128, 8, 256] fp16 tile ([:, :, :D] msg,
   [:, :, D:] gates) so the wide mm2 reads one contiguous 256-col rhs.
 - The fp16 msg*gates multiply is split within each chunk: 2-3 tiles on
   DVE (2x mode) and the rest on GPSIMD (Pool engine, via engine-field
   retarget of a vector tensor_mul; CoreSim+HW verified exact) in two
   ops aligned with the sigmoid halves. GPSIMD absorbs ~40% of the
   multiply work that would otherwise saturate DVE.
 - mm2 of chunk c is emitted ~7 chunks later (in half-chunk units) in
   the PE stream so the in-order PE is never stalled by the multiply
   latency; the [B,2D] PSUM accumulator makes this safe.
 - Every dma_start costs ~650ns of serial dispatch on the SP queue, so
   all fp16 constants ship in three packed slices of one [128, 3456]
   tensor (weights/bias rows first), and mask-slab DMAs trail node
   slabs by one slab (mm2 runs 7 chunks behind, so masks are never
   urgent). First compute starts ~3us earlier than with per-const DMAs.
Host side: inputs are cast/transposed to fp16 ([S,N] nodes, [128 nodes,
tile, B] masks); biases are exact to fp16 rounding (~6e-5), overall
rel err vs the fp32 reference ~1.2e-4.
"""

import json

import numpy as np

import concourse.bass as bass
import concourse.mybir as mybir
import concourse.tile as tile
from concourse import bass2jax as _b2j
from concourse import bass_utils as _bu
from concourse.bass_utils import run_bass_kernel_spmd


def _split_excess_waits_json(bir_json) -> bytes:
    """Walrus in this container accepts at most 1 embedded sem-wait per
    instruction (2 for EventSemaphore). Tile emits instructions (notably the
    kernel-tail Drain) with more. Move excess waits onto injected
    EventSemaphore instructions placed immediately before the offender in
    the same engine stream — identical blocking semantics."""
    if isinstance(bir_json, str):
        bir_json = bir_json.encode()
    d = json.loads(bir_json)
    counter = [0]

    def fix_block(b):
        new = []
        for inst in b.get("instructions", []):
            si = inst.get("sync_info")
            waits = (si or {}).get("on_wait") or []
            cap = 2 if inst.get("opcode") == "EventSemaphore" else 1
            if len(waits) > cap:
                keep, excess = waits[:cap], waits[cap:]
                for j in range(0, len(excess), 2):
                    counter[0] += 1
                    new.append(
                        {
                            "debug": inst.get("debug"),
                            "engine": inst["engine"],
                            "ins": [],
                            "outs": [],
                            "name": f"antsplit_ev_{counter[0]}",
                            "opcode": "EventSemaphore",
                            "sync_info": {
                                "on_update": [],
                                "on_wait": excess[j : j + 2],
                            },
                        }
                    )
                si["on_wait"] = keep
            new.append(inst)
        b["instructions"] = new
        for sb in b.get("blocks", []):
            fix_block(sb)

    for f in d.get("functions", []):
        for blk in f.get("blocks", []):
            fix_block(blk)
    return json.dumps(d).encode()


if not getattr(_bu, "_ant_split_waits_patched", False):
    _orig_compile_bir_kernel = _bu.compile_bir_kernel

    def _patched_compile_bir_kernel(bir_json, tmpdir, neff_name="file.neff"):
        return _orig_compile_bir_kernel(
            _split_excess_waits_json(bir_json), tmpdir, neff_name
        )

    _bu.compile_bir_kernel = _patched_compile_bir_kernel
    _b2j.compile_bir_kernel = _patched_compile_bir_kernel
    _bu._ant_split_waits_patched = True

N_CORES = 8
N_TOTAL = 500_000
B = 128
S = 128
D = 128
P = 128

N_PER_CORE = N_TOTAL // N_CORES          # 62500
TILES_PER_CHUNK = 8
CHUNK = TILES_PER_CHUNK * P              # 1024
SLAB_CHUNKS = [1, 1, 2] + [4] * 14 + [2]  # 62 chunks
N_CHUNKS = sum(SLAB_CHUNKS)              # 62
N_TILES = N_CHUNKS * TILES_PER_CHUNK     # 496
N_PAD = N_TILES * P                      # 63488

F16 = mybir.dt.float16
F32 = mybir.dt.float32
NP_F16 = np.float16

# packed fp16 const layout (one [P, CW16] dram tensor / SBUF tile):
#   cols 0:1024       btb16 (bt tiled, all partitions)
#   cols 1024:1152    wt
#   cols 1152:1280    wg
#   cols 1280:2304    row p=0: bgbr (bg tiled)
#   cols 2304:3328    bgb16 (bg tiled, all partitions)
#   cols 3328:3456    row p=0: ones
CW16 = 3456

OPTS = {
    "sigmoid": True,
    "warm_mms": 0,
    "touches": True,
    # tiles of each chunk's multiply that run on DVE; the rest go to the
    # Pool (GPSIMD) engine
    "mul_dve_tiles": [3, 2, 2],
    "tail_chunks": 3,
    "d_first": True,
    "mm2_delay": 7,
}

# per-chunk mode cycles (index = c % len):
#  d: "tt" = DVE fused bias-evict; "cp" = W-type (ACT copy evict,
#     256-wide mm2, host bt fix)
#  g: "r1" = rank-1 PE matmul bias; "tt" = DVE tensor_add bias
D_CYCLE = ["cp"] + ["tt"] * 23
G_CYCLE = ["r1", "r1", "r1", "r1", "tt"]


def g_mode(c):
    if c == 0:
        return "r1"
    if c >= N_CHUNKS - OPTS["tail_chunks"]:
        return "r1"
    return G_CYCLE[c % len(G_CYCLE)]


def d_mode(c):
    if c == 0:
        return "cp"  # first mm2 (256 wide) initializes the whole pool12
    if c >= N_CHUNKS - OPTS["tail_chunks"]:
        return "tt"
    return D_CYCLE[c % len(D_CYCLE)]


def mul_dve_tiles(c):
    if c >= N_CHUNKS - OPTS["tail_chunks"]:
        return TILES_PER_CHUNK  # tail: all-DVE multiply, short latency
    return OPTS["mul_dve_tiles"][c % len(OPTS["mul_dve_tiles"])]


def build_bass() -> bass.Bass:
    assert sum(SLAB_CHUNKS) == N_CHUNKS
    nc = bass.Bass()

    nodesT = nc.dram_tensor("nodesT", [P, N_PAD], F16, kind="ExternalInput").ap()
    masksT = nc.dram_tensor("masksT", [P, N_TILES, B], F16, kind="ExternalInput").ap()
    c16_d = nc.dram_tensor("c16", [P, CW16], F16, kind="ExternalInput").ap()
    out_d = nc.dram_tensor("out", [B, 2 * D], F32, kind="ExternalOutput").ap()

    def pool_mul(out, in0, in1):
        inst = nc.vector.tensor_mul(out=out, in0=in0, in1=in1)
        inst.ins.engine = mybir.EngineType.Pool
        return inst

    nslabs = len(SLAB_CHUNKS)
    slab_off = [0] * nslabs
    off = 0
    for s, sc in enumerate(SLAB_CHUNKS):
        slab_off[s] = off
        off += sc * CHUNK

    with tile.TileContext(nc) as tc:
        with (
            tc.tile_pool(name="consts", bufs=1) as consts,
            tc.tile_pool(name="scratch", bufs=1) as scratch,
            tc.tile_pool(name="nodes", bufs=4) as nodes_pool,
            tc.tile_pool(name="masks", bufs=6) as masks_pool,
            tc.tile_pool(name="gpre", bufs=2) as gpre_pool,
            tc.tile_pool(name="mg", bufs=9) as mg_pool,
            tc.tile_pool(name="outs", bufs=1) as out_pool,
            tc.tile_pool(name="psd", bufs=2, space="PSUM") as psd_pool,
            tc.tile_pool(name="psg", bufs=3, space="PSUM") as psg_pool,
            tc.tile_pool(name="acc", bufs=1, space="PSUM") as acc_pool,
        ):
            def nodes_tile():
                return nodes_pool.tile(
                    [P, 4 * CHUNK], F16, tag="nod_slab", name="nod_slab"
                )

            def masks_tile():
                return masks_pool.tile(
                    [P, 4 * TILES_PER_CHUNK, B], F16,
                    tag="mk_slab", name="mk_slab",
                )

            def emit_nodes_dma(tile_, s):
                nc.sync.dma_start(
                    tile_[:, : SLAB_CHUNKS[s] * CHUNK],
                    nodesT[:, slab_off[s] : slab_off[s] + SLAB_CHUNKS[s] * CHUNK],
                )

            def emit_masks_dma(tile_, s):
                to = slab_off[s] // P
                nt = SLAB_CHUNKS[s] * TILES_PER_CHUNK
                nc.sync.dma_start(tile_[:, :nt, :], masksT[:, to : to + nt, :])

            # ---- startup DMA queue: wt/wg/bgbr/ones block, first nodes,
            # remaining consts; masks trail nodes by one slab ----
            c16_sb = consts.tile([P, CW16], F16)
            nc.sync.dma_start(c16_sb[:, 1024:2304], c16_d[:, 1024:2304])  # wt|wg|bgbr
            nod_slabs = [nodes_tile()]
            emit_nodes_dma(nod_slabs[0], 0)
            nc.sync.dma_start(c16_sb[:, :1024], c16_d[:, :1024])
            nc.sync.dma_start(c16_sb[:, 2304:], c16_d[:, 2304:])  # bgb16+ones

            btb16_sb = c16_sb[:, 0:1024]
            wt_sb = c16_sb[:, 1024:1152]
            wg_sb = c16_sb[:, 1152:1280]
            bgbr_sb = c16_sb[0:1, 1280:2304]
            ones_sb = c16_sb[0:1, 3328:3456]
            bgb16_sb = c16_sb[:, 2304:3328]

            # One-time const touches: absorb the const-DMA semaphores into
            # each engine's observed clock so hot-loop instructions never
            # need a second (DMA) wait slot.
            if OPTS["touches"]:
                dve_scratch = scratch.tile([1, 4], F32)
                nc.vector.tensor_copy(
                    out=dve_scratch[:1, :1], in_=btb16_sb[:1, :1]
                )
                nc.vector.tensor_copy(
                    out=dve_scratch[:1, 1:2], in_=bgb16_sb[:1, :1]
                )
                pool_scr = scratch.tile([1, 4], F16, tag="pscr")
                pool_mul(pool_scr[:1, :1], bgbr_sb[:1, :1], bgbr_sb[:1, :1])
                nc.tensor.ldweights(wt_sb[:, :1])
                nc.tensor.ldweights(wg_sb[:, :1])
                nc.tensor.ldweights(ones_sb[:, :1])
            if OPTS["warm_mms"]:
                # burn the PE clock ramp during the initial DMA wait
                warm_ps = psg_pool.tile([P, CHUNK // 2], F32, tag="psg")
                for _ in range(OPTS["warm_mms"]):
                    nc.tensor.matmul(
                        warm_ps[:, :D], wt_sb[:], wg_sb[:], start=True, stop=True
                    )
                nc.vector.tensor_copy(
                    out=dve_scratch[:1, 2:3], in_=warm_ps[:1, :1]
                )

            # pool12[:, :D] accumulates M@msg; [:, D:] accumulates M@G for
            # W-type chunks only (host multiplies by bt). Chunk 0 is W-type
            # and its first 256-wide mm2 carries start=True for the whole
            # [B, 2D] region.
            pool12 = acc_pool.tile([B, 2 * D], F32)

            def emit_mm2(mk, mg_t, cc, h):
                # one half-chunk of pooling matmuls: depends only on its own
                # multiply half, so PE interleaves finer at chunk boundaries
                wide = d_mode(cc) == "cp"
                HT2 = TILES_PER_CHUNK // 2
                for t in range(h * HT2, (h + 1) * HT2):
                    first = cc == 0 and t == 0
                    last = cc == N_CHUNKS - 1 and t == TILES_PER_CHUNK - 1
                    nc.tensor.matmul(
                        pool12[:] if wide else pool12[:, :D],
                        mk[:, t, :],
                        mg_t[:, t, :] if wide else mg_t[:, t, :D],
                        start=first,
                        stop=last,
                        skip_group_check=True,
                    )

            pending_mm2 = []  # [(mk, mg_t, c, half), ...]
            c = 0
            mk_slabs = [masks_tile()]
            for s in range(nslabs):
                # prefetch next slab's nodes; this slab's masks DMA
                # (mask tiles pre-allocated so chunk code can reference them)
                if s + 1 < nslabs:
                    nt = nodes_tile()
                    emit_nodes_dma(nt, s + 1)
                    nod_slabs.append(nt)
                emit_masks_dma(mk_slabs[s], s)
                if s + 1 < nslabs:
                    mk_slabs.append(masks_tile())
                nod_slab = nod_slabs[s]
                mk_slab = mk_slabs[s]

                for cs in range(SLAB_CHUNKS[s]):
                    nod = nod_slab[:, cs * CHUNK : (cs + 1) * CHUNK]
                    mk = mk_slab[
                        :, cs * TILES_PER_CHUNK : (cs + 1) * TILES_PER_CHUNK, :
                    ]
                    gm = g_mode(c)
                    dm = d_mode(c)

                    H = CHUNK // 2
                    HT = TILES_PER_CHUNK // 2
                    psum_d = psd_pool.tile([P, CHUNK], F32, tag="psd")
                    pg0 = psg_pool.tile([P, H], F32, tag="psg")
                    pg1 = psg_pool.tile([P, H], F32, tag="psg")
                    psum_g = [pg0, pg1]
                    if gm == "r1":
                        for pg in psum_g:
                            nc.tensor.matmul(
                                pg[:],
                                ones_sb[:],
                                bgbr_sb[:, :H],
                                start=True,
                                stop=False,
                                skip_group_check=True,
                            )
                    for t in range(TILES_PER_CHUNK):
                        sl = bass.ts(t, P)

                        def mmd():
                            nc.tensor.matmul(
                                psum_d[:, sl],
                                nod[:, sl],
                                wt_sb[:],
                                start=True,
                                stop=True,
                            )

                        def mmg():
                            nc.tensor.matmul(
                                psum_g[t // HT][:, bass.ts(t % HT, P)],
                                nod[:, sl],
                                wg_sb[:],
                                start=gm != "r1",
                                stop=True,
                                skip_group_check=gm == "r1",
                            )

                        if OPTS["d_first"]:
                            mmd()
                            mmg()
                        else:
                            mmg()
                            mmd()

                    # an earlier chunk's pooling matmuls go here in the PE
                    # stream: its multiply gets several chunks of mm1 slack
                    while len(pending_mm2) >= 2 * OPTS["mm2_delay"]:
                        emit_mm2(*pending_mm2.pop(0))
                        emit_mm2(*pending_mm2.pop(0))

                    mg_t = mg_pool.tile(
                        [P, TILES_PER_CHUNK, 2 * D], F16, tag="mg"
                    )
                    sig = (
                        mybir.ActivationFunctionType.Sigmoid
                        if OPTS["sigmoid"]
                        else mybir.ActivationFunctionType.Copy
                    )
                    psum_d3 = psum_d.rearrange("p (t d) -> p t d", d=D)
                    # data-path eviction first when it runs on ACT (in-order
                    # engine; psum_d is ready before the gates matmuls)
                    if dm == "cp":
                        nc.scalar.copy(out=mg_t[:, :, :D], in_=psum_d3[:])
                    # gates into mg[:, :, D:]
                    if gm == "r1":
                        for h, pg in enumerate(psum_g):
                            nc.scalar.activation(
                                mg_t[:, h * HT : (h + 1) * HT, D:],
                                pg.rearrange("p (t d) -> p t d", d=D),
                                sig,
                            )
                    else:
                        gpre_t = gpre_pool.tile([P, CHUNK], F16, tag="gpre")
                        for h, pg in enumerate(psum_g):
                            nc.vector.tensor_add(
                                out=gpre_t[:, h * H : (h + 1) * H],
                                in0=pg[:],
                                in1=bgb16_sb[:, :H],
                            )
                        nc.scalar.activation(
                            mg_t[:, :, D:],
                            gpre_t.rearrange("p (t d) -> p t d", d=D),
                            sig,
                        )
                    # data into mg[:, :, :D] (cp: copied above, host bt fix)
                    if dm == "tt":
                        nc.vector.tensor_add(
                            out=mg_t[:, :, :D],
                            in0=psum_d3[:],
                            in1=btb16_sb.rearrange("p (t d) -> p t d", d=D),
                        )
                    # multiply msg = a * g, split DVE/Pool within the chunk
                    ndve = mul_dve_tiles(c)
                    if ndve > 0:
                        nc.vector.tensor_mul(
                            out=mg_t[:, :ndve, :D],
                            in0=mg_t[:, :ndve, :D],
                            in1=mg_t[:, :ndve, D:],
                        )
                    if ndve < HT:
                        # two Pool ops aligned with the sigmoid halves so
                        # each starts as soon as its gates half is ready
                        pool_mul(
                            mg_t[:, ndve:HT, :D],
                            mg_t[:, ndve:HT, :D],
                            mg_t[:, ndve:HT, D:],
                        )
                    if ndve < TILES_PER_CHUNK:
                        pool_mul(
                            mg_t[:, max(ndve, HT) :, :D],
                            mg_t[:, max(ndve, HT) :, :D],
                            mg_t[:, max(ndve, HT) :, D:],
                        )

                    pending_mm2.append((mk, mg_t, c, 0))
                    pending_mm2.append((mk, mg_t, c, 1))
                    c += 1

            for pm in pending_mm2:
                emit_mm2(*pm)
            pending_mm2 = []

            res = out_pool.tile([B, 2 * D], F32)
            nc.vector.tensor_copy(out=res[:], in_=pool12[:])
            nc.sync.dma_start(out_d, res[:])

    return nc


_CACHE: dict = {}


def _get_bass() -> bass.Bass:
    if "nc" not in _CACHE:
        _CACHE["nc"] = build_bass()
    return _CACHE["nc"]


def _prepare_in_maps(nodes, owner_masks, Wt, bt, Wg, bg):
    nodes_h = np.asarray(nodes, dtype=NP_F16)
    masks = np.asarray(owner_masks)
    wt_h = np.asarray(Wt, dtype=NP_F16)
    wg_h = np.asarray(Wg, dtype=NP_F16)
    bt16 = np.asarray(bt, dtype=NP_F16)
    bg16 = np.asarray(bg, dtype=NP_F16)

    c16 = np.zeros((P, CW16), dtype=NP_F16)
    c16[:, 0:1024] = np.tile(bt16[None, :], (P, CHUNK // D))
    c16[:, 1024:1152] = wt_h
    c16[:, 1152:1280] = wg_h
    c16[0, 1280:2304] = np.tile(bg16, CHUNK // D)
    c16[0, 3328:3456] = 1.0
    c16[:, 2304:3328] = np.tile(bg16[None, :], (P, CHUNK // D))

    in_maps = []
    for core in range(N_CORES):
        off = core * N_PER_CORE
        ncr = np.zeros((P, N_PAD), dtype=NP_F16)
        ncr[:, :N_PER_CORE] = nodes_h[off : off + N_PER_CORE].T
        mp = np.zeros((B, N_PAD), dtype=NP_F16)
        mp[:, :N_PER_CORE] = masks[:, off : off + N_PER_CORE]
        mkt = np.ascontiguousarray(mp.reshape(B, N_TILES, P).transpose(2, 1, 0))
        in_maps.append(
            {
                "nodesT": ncr,
                "masksT": mkt,
                "c16": c16,
            }
        )
    return in_maps


def run(inputs: dict, trace: bool = False):
    """Run the kernel. Returns (pooled [B, D] float32, BassKernelResults)."""
    nc = _get_bass()
    in_maps = _prepare_in_maps(**inputs)
    rb = run_bass_kernel_spmd(
        nc, in_maps, core_ids=list(range(N_CORES)), trace=trace
    )
    parts = np.stack([r["out"].astype(np.float64) for r in rb.results])
    tot = parts.sum(axis=0)
    bt64 = np.asarray(inputs["bt"], dtype=np.float64)
    pooled = tot[:, :D] + tot[:, D:] * bt64[None, :]
    return pooled.astype(np.float32), rb


def kernel(**inputs) -> np.ndarray:
    try:
        out, _ = run(inputs, trace=False)
    except Exception:
        # transient device errors (e.g. residual bad state from a previous
        # crashed NEFF) have been observed once; one retry clears them
        out, _ = run(inputs, trace=False)
    return out


if __name__ == "__main__":
    rng = np.random.default_rng(0)
    demo = {
        "nodes": rng.standard_normal((N_TOTAL, S), dtype=np.float32),
        "owner_masks": rng.integers(0, 2, (B, N_TOTAL)).astype(np.int32),
        "Wt": rng.standard_normal((S, D), dtype=np.float32) * 0.09,
        "bt": rng.standard_normal(D).astype(np.float32) * 0.09,
        "Wg": rng.standard_normal((S, D), dtype=np.float32) * 0.09,
        "bg": rng.standard_normal(D).astype(np.float32) * 0.09,
    }
    out = kernel(**demo)
    print(out.shape, out.dtype, np.abs(out).mean())


# revision 43
# speedup vs baseline: 1.0074x; 1.0074x over previous
"""Trainium2 Bass kernel for nn_Aggregator (gnn_message_passing).

pooled[B,D] = owner_masks.f32 @ ((nodes@Wt + bt) * sigmoid(nodes@Wg + bg))

Sharding: nodes (and owner_masks columns) split along N across 8 cores;
the host sums the 8 partial results and applies the small bt correction
for "W-type" chunks (see below).

Design (cost-model exec ~115.9us vs 141.5us for the v1 kernel; engine
busy: PE ~105, DVE ~99, Pool ~94, DMA ~93, ACT ~78):
 - Per chunk (8 tiles of 128 nodes): 16 mm1 matmuls -> psum_d [128,1024]
   and psum_g split into two [128,512] half-tiles. Asymmetric PSUM pools
   (psum_d 2 bufs x 2 banks, psum_g halves 3 bufs x 1 bank, [B,2D]
   accumulator 1 bank = 8 banks) give 2 chunks of rotation lookahead;
   symmetric 3-slot pools stalled ~1us/chunk on the rotation edges.
 - The gates bias bg enters psum_g via rank-1 K=1 PE matmuls
   (ones[1,128].T @ bgb_row, 2x213ns) on ~80% of chunks; the rest use a
   DVE tensor_add into a gpre tile to balance PE vs DVE (the g-bias is
   the only work that can move between those two engines). ACT reads
   sigmoid straight from PSUM on rank-1 chunks, one op per half, so
   each psum_g half frees early.
 - The data bias bt rides free on the psum_d eviction: one DVE
   tensor_tensor add (psum fp32 + fp16 btb -> fp16 msg). W-type chunks
   (3 of 62) instead evict with an ACT copy (no bias) and run a
   256-wide mm2 over [msg|G], accumulating M@G into pool12[:, D:]; the
   host applies pooled += (M@G)_W * bt exactly. Chunk 0 is W-type and
   its first wide mm2 (start=True) initializes the whole accumulator.
 - msg and gates share a [128, 8, 256] fp16 tile ([:, :, :D] msg,
   [:, :, D:] gates) so the wide mm2 reads one contiguous 256-col rhs.
 - The fp16 msg*gates multiply is split within each chunk: 2-3 tiles on
   DVE (2x mode) and the rest on GPSIMD (Pool engine, via engine-field
   retarget of a vector tensor_mul; CoreSim+HW verified exact) in two
   ops aligned with the sigmoid halves. GPSIMD absorbs ~40% of the
   multiply work that would otherwise saturate DVE.
 - mm2 of chunk c is emitted ~7 chunks later (in half-chunk units) in
   the PE stream so the in-order PE is never stalled by the multiply
   latency; the [B,2D] PSUM accumulator makes this safe.
 - Every dma_start costs ~650ns of serial dispatch on the SP queue, so
   all fp16 constants ship in three packed slices of one [128, 3456]
   tensor (weights/bias rows first), and mask-slab DMAs trail node
   slabs by one slab (mm2 runs 7 chunks behind, so masks are never
   urgent). First compute starts ~3us earlier than with per-const DMAs.
Host side: inputs are cast/transposed to fp16 ([S,N] nodes, [128 nodes,
tile, B] masks); biases are exact to fp16 rounding (~6e-5), overall
rel err vs the fp32 reference ~1.2e-4.
"""

import json

import numpy as np

import concourse.bass as bass
import concourse.mybir as mybir
import concourse.tile as tile
from concourse import bass2jax as _b2j
from concourse import bass_utils as _bu
from concourse.bass_utils import run_bass_kernel_spmd


def _split_excess_waits_json(bir_json) -> bytes:
    """Walrus in this container accepts at most 1 embedded sem-wait per
    instruction (2 for EventSemaphore). Tile emits instructions (notably the
    kernel-tail Drain) with more. Move excess waits onto injected
    EventSemaphore instructions placed immediately before the offender in
    the same engine stream — identical blocking semantics."""
    if isinstance(bir_json, str):
        bir_json = bir_json.encode()
    d = json.loads(bir_json)
    counter = [0]

    def fix_block(b):
        new = []
        for inst in b.get("instructions", []):
            si = inst.get("sync_info")
            waits = (si or {}).get("on_wait") or []
            cap = 2 if inst.get("opcode") == "EventSemaphore" else 1
            if len(waits) > cap:
                keep, excess = waits[:cap], waits[cap:]
                for j in range(0, len(excess), 2):
                    counter[0] += 1
                    new.append(
                        {
                            "debug": inst.get("debug"),
                            "engine": inst["engine"],
                            "ins": [],
                            "outs": [],
                            "name": f"antsplit_ev_{counter[0]}",
                            "opcode": "EventSemaphore",
                            "sync_info": {
                                "on_update": [],
                                "on_wait": excess[j : j + 2],
                            },
                        }
                    )
                si["on_wait"] = keep
            new.append(inst)
        b["instructions"] = new
        for sb in b.get("blocks", []):
            fix_block(sb)

    for f in d.get("functions", []):
        for blk in f.get("blocks", []):
            fix_block(blk)
    return json.dumps(d).encode()


if not getattr(_bu, "_ant_split_waits_patched", False):
    _orig_compile_bir_kernel = _bu.compile_bir_kernel

    def _patched_compile_bir_kernel(bir_json, tmpdir, neff_name="file.neff"):
        return _orig_compile_bir_kernel(
            _split_excess_waits_json(bir_json), tmpdir, neff_name
        )

    _bu.compile_bir_kernel = _patched_compile_bir_kernel
    _b2j.compile_bir_kernel = _patched_compile_bir_kernel
    _bu._ant_split_waits_patched = True

N_CORES = 8
N_TOTAL = 500_000
B = 128
S = 128
D = 128
P = 128

N_PER_CORE = N_TOTAL // N_CORES          # 62500
TILES_PER_CHUNK = 8
CHUNK = TILES_PER_CHUNK * P              # 1024
SLAB_CHUNKS = [2] * 31  # 62 chunks
N_CHUNKS = sum(SLAB_CHUNKS)              # 62
N_TILES = N_CHUNKS * TILES_PER_CHUNK     # 496
N_PAD = N_TILES * P                      # 63488

F16 = mybir.dt.float16
F32 = mybir.dt.float32
NP_F16 = np.float16

# packed fp16 const layout (one [P, CW16] dram tensor / SBUF tile):
#   cols 0:1024       btb16 (bt tiled, all partitions)
#   cols 1024:1152    wt
#   cols 1152:1280    wg
#   cols 1280:2304    row p=0: bgbr (bg tiled)
#   cols 2304:3328    bgb16 (bg tiled, all partitions)
#   cols 3328:3456    row p=0: ones
CW16 = 3456

OPTS = {
    "sigmoid": True,
    "warm_mms": 0,
    "touches": True,
    # tiles of each chunk's multiply that run on DVE; the rest go to the
    # Pool (GPSIMD) engine
    "mul_dve_tiles": [3, 2, 2],
    "tail_chunks": 3,
    "d_first": True,
    "mm2_delay": 7,
}

# per-chunk mode cycles (index = c % len):
#  d: "tt" = DVE fused bias-evict; "cp" = W-type (ACT copy evict,
#     256-wide mm2, host bt fix)
#  g: "r1" = rank-1 PE matmul bias; "tt" = DVE tensor_add bias
D_CYCLE = ["cp"] + ["tt"] * 23
G_CYCLE = ["r1", "r1", "r1", "r1", "tt"]


def g_mode(c):
    if c == 0:
        return "r1"
    if c >= N_CHUNKS - OPTS["tail_chunks"]:
        return "r1"
    return G_CYCLE[c % len(G_CYCLE)]


def d_mode(c):
    if c == 0:
        return "cp"  # first mm2 (256 wide) initializes the whole pool12
    if c >= N_CHUNKS - OPTS["tail_chunks"]:
        return "tt"
    return D_CYCLE[c % len(D_CYCLE)]


def mul_dve_tiles(c):
    if c >= N_CHUNKS - OPTS["tail_chunks"]:
        return TILES_PER_CHUNK  # tail: all-DVE multiply, short latency
    return OPTS["mul_dve_tiles"][c % len(OPTS["mul_dve_tiles"])]


def build_bass() -> bass.Bass:
    assert sum(SLAB_CHUNKS) == N_CHUNKS
    nc = bass.Bass()

    nodesT = nc.dram_tensor("nodesT", [P, N_PAD], F16, kind="ExternalInput").ap()
    masksT = nc.dram_tensor("masksT", [P, N_TILES, B], F16, kind="ExternalInput").ap()
    c16_d = nc.dram_tensor("c16", [P, CW16], F16, kind="ExternalInput").ap()
    out_d = nc.dram_tensor("out", [B, 2 * D], F32, kind="ExternalOutput").ap()

    def pool_mul(out, in0, in1):
        inst = nc.vector.tensor_mul(out=out, in0=in0, in1=in1)
        inst.ins.engine = mybir.EngineType.Pool
        return inst

    nslabs = len(SLAB_CHUNKS)
    slab_off = [0] * nslabs
    off = 0
    for s, sc in enumerate(SLAB_CHUNKS):
        slab_off[s] = off
        off += sc * CHUNK

    with tile.TileContext(nc) as tc:
        with (
            tc.tile_pool(name="consts", bufs=1) as consts,
            tc.tile_pool(name="scratch", bufs=1) as scratch,
            tc.tile_pool(name="nodes", bufs=6) as nodes_pool,
            tc.tile_pool(name="masks", bufs=8) as masks_pool,
            tc.tile_pool(name="gpre", bufs=2) as gpre_pool,
            tc.tile_pool(name="mg", bufs=9) as mg_pool,
            tc.tile_pool(name="outs", bufs=1) as out_pool,
            tc.tile_pool(name="psd", bufs=2, space="PSUM") as psd_pool,
            tc.tile_pool(name="psg", bufs=3, space="PSUM") as psg_pool,
            tc.tile_pool(name="acc", bufs=1, space="PSUM") as acc_pool,
        ):
            def nodes_tile():
                return nodes_pool.tile(
                    [P, 2 * CHUNK], F16, tag="nod_slab", name="nod_slab"
                )

            def masks_tile():
                return masks_pool.tile(
                    [P, 2 * TILES_PER_CHUNK, B], F16,
                    tag="mk_slab", name="mk_slab",
                )

            def emit_nodes_dma(tile_, s):
                nc.sync.dma_start(
                    tile_[:, : SLAB_CHUNKS[s] * CHUNK],
                    nodesT[:, slab_off[s] : slab_off[s] + SLAB_CHUNKS[s] * CHUNK],
                )

            def emit_masks_dma(tile_, s):
                to = slab_off[s] // P
                nt = SLAB_CHUNKS[s] * TILES_PER_CHUNK
                nc.sync.dma_start(tile_[:, :nt, :], masksT[:, to : to + nt, :])

            # ---- startup DMA queue: wt/wg/bgbr/ones block, first nodes,
            # remaining consts; masks trail nodes by one slab ----
            c16_sb = consts.tile([P, CW16], F16)
            nc.sync.dma_start(c16_sb[:, 1024:2304], c16_d[:, 1024:2304])  # wt|wg|bgbr
            nod_slabs = [nodes_tile()]
            emit_nodes_dma(nod_slabs[0], 0)
            nc.sync.dma_start(c16_sb[:, :1024], c16_d[:, :1024])
            nc.sync.dma_start(c16_sb[:, 2304:], c16_d[:, 2304:])  # bgb16+ones

            btb16_sb = c16_sb[:, 0:1024]
            wt_sb = c16_sb[:, 1024:1152]
            wg_sb = c16_sb[:, 1152:1280]
            bgbr_sb = c16_sb[0:1, 1280:2304]
            ones_sb = c16_sb[0:1, 3328:3456]
            bgb16_sb = c16_sb[:, 2304:3328]

            # One-time const touches: absorb the const-DMA semaphores into
            # each engine's observed clock so hot-loop instructions never
            # need a second (DMA) wait slot.
            if OPTS["touches"]:
                dve_scratch = scratch.tile([1, 4], F32)
                nc.vector.tensor_copy(
                    out=dve_scratch[:1, :1], in_=btb16_sb[:1, :1]
                )
                nc.vector.tensor_copy(
                    out=dve_scratch[:1, 1:2], in_=bgb16_sb[:1, :1]
                )
                pool_scr = scratch.tile([1, 4], F16, tag="pscr")
                pool_mul(pool_scr[:1, :1], bgbr_sb[:1, :1], bgbr_sb[:1, :1])
                nc.tensor.ldweights(wt_sb[:, :1])
                nc.tensor.ldweights(wg_sb[:, :1])
                nc.tensor.ldweights(ones_sb[:, :1])
            if OPTS["warm_mms"]:
                # burn the PE clock ramp during the initial DMA wait
                warm_ps = psg_pool.tile([P, CHUNK // 2], F32, tag="psg")
                for _ in range(OPTS["warm_mms"]):
                    nc.tensor.matmul(
                        warm_ps[:, :D], wt_sb[:], wg_sb[:], start=True, stop=True
                    )
                nc.vector.tensor_copy(
                    out=dve_scratch[:1, 2:3], in_=warm_ps[:1, :1]
                )

            # pool12[:, :D] accumulates M@msg; [:, D:] accumulates M@G for
            # W-type chunks only (host multiplies by bt). Chunk 0 is W-type
            # and its first 256-wide mm2 carries start=True for the whole
            # [B, 2D] region.
            pool12 = acc_pool.tile([B, 2 * D], F32)

            def emit_mm2(mk, mg_t, cc, h):
                # one half-chunk of pooling matmuls: depends only on its own
                # multiply half, so PE interleaves finer at chunk boundaries
                wide = d_mode(cc) == "cp"
                HT2 = TILES_PER_CHUNK // 2
                for t in range(h * HT2, (h + 1) * HT2):
                    first = cc == 0 and t == 0
                    last = cc == N_CHUNKS - 1 and t == TILES_PER_CHUNK - 1
                    nc.tensor.matmul(
                        pool12[:] if wide else pool12[:, :D],
                        mk[:, t, :],
                        mg_t[:, t, :] if wide else mg_t[:, t, :D],
                        start=first,
                        stop=last,
                        skip_group_check=True,
                    )

            pending_mm2 = []  # [(mk, mg_t, c, half), ...]
            c = 0
            mk_slabs = [masks_tile()]
            for s in range(nslabs):
                # prefetch next slab's nodes; this slab's masks DMA
                # (mask tiles pre-allocated so chunk code can reference them)
                if s + 1 < nslabs:
                    nt = nodes_tile()
                    emit_nodes_dma(nt, s + 1)
                    nod_slabs.append(nt)
                emit_masks_dma(mk_slabs[s], s)
                if s + 1 < nslabs:
                    mk_slabs.append(masks_tile())
                nod_slab = nod_slabs[s]
                mk_slab = mk_slabs[s]

                for cs in range(SLAB_CHUNKS[s]):
                    nod = nod_slab[:, cs * CHUNK : (cs + 1) * CHUNK]
                    mk = mk_slab[
                        :, cs * TILES_PER_CHUNK : (cs + 1) * TILES_PER_CHUNK, :
                    ]
                    gm = g_mode(c)
                    dm = d_mode(c)

                    H = CHUNK // 2
                    HT = TILES_PER_CHUNK // 2
                    psum_d = psd_pool.tile([P, CHUNK], F32, tag="psd")
                    pg0 = psg_pool.tile([P, H], F32, tag="psg")
                    pg1 = psg_pool.tile([P, H], F32, tag="psg")
                    psum_g = [pg0, pg1]
                    if gm == "r1":
                        for pg in psum_g:
                            nc.tensor.matmul(
                                pg[:],
                                ones_sb[:],
                                bgbr_sb[:, :H],
                                start=True,
                                stop=False,
                                skip_group_check=True,
                            )
                    for t in range(TILES_PER_CHUNK):
                        sl = bass.ts(t, P)

                        def mmd():
                            nc.tensor.matmul(
                                psum_d[:, sl],
                                nod[:, sl],
                                wt_sb[:],
                                start=True,
                                stop=True,
                            )

                        def mmg():
                            nc.tensor.matmul(
                                psum_g[t // HT][:, bass.ts(t % HT, P)],
                                nod[:, sl],
                                wg_sb[:],
                                start=gm != "r1",
                                stop=True,
                                skip_group_check=gm == "r1",
                            )

                        if OPTS["d_first"]:
                            mmd()
                            mmg()
                        else:
                            mmg()
                            mmd()

                    # an earlier chunk's pooling matmuls go here in the PE
                    # stream: its multiply gets several chunks of mm1 slack
                    while len(pending_mm2) >= 2 * OPTS["mm2_delay"]:
                        emit_mm2(*pending_mm2.pop(0))
                        emit_mm2(*pending_mm2.pop(0))

                    mg_t = mg_pool.tile(
                        [P, TILES_PER_CHUNK, 2 * D], F16, tag="mg"
                    )
                    sig = (
                        mybir.ActivationFunctionType.Sigmoid
                        if OPTS["sigmoid"]
                        else mybir.ActivationFunctionType.Copy
                    )
                    psum_d3 = psum_d.rearrange("p (t d) -> p t d", d=D)
                    # data-path eviction first when it runs on ACT (in-order
                    # engine; psum_d is ready before the gates matmuls)
                    if dm == "cp":
                        nc.scalar.copy(out=mg_t[:, :, :D], in_=psum_d3[:])
                    # gates into mg[:, :, D:]
                    if gm == "r1":
                        for h, pg in enumerate(psum_g):
                            nc.scalar.activation(
                                mg_t[:, h * HT : (h + 1) * HT, D:],
                                pg.rearrange("p (t d) -> p t d", d=D),
                                sig,
                            )
                    else:
                        gpre_t = gpre_pool.tile([P, CHUNK], F16, tag="gpre")
                        for h, pg in enumerate(psum_g):
                            nc.vector.tensor_add(
                                out=gpre_t[:, h * H : (h + 1) * H],
                                in0=pg[:],
                                in1=bgb16_sb[:, :H],
                            )
                        nc.scalar.activation(
                            mg_t[:, :, D:],
                            gpre_t.rearrange("p (t d) -> p t d", d=D),
                            sig,
                        )
                    # data into mg[:, :, :D] (cp: copied above, host bt fix)
                    if dm == "tt":
                        nc.vector.tensor_add(
                            out=mg_t[:, :, :D],
                            in0=psum_d3[:],
                            in1=btb16_sb.rearrange("p (t d) -> p t d", d=D),
                        )
                    # multiply msg = a * g, split DVE/Pool within the chunk
                    ndve = mul_dve_tiles(c)
                    if ndve > 0:
                        nc.vector.tensor_mul(
                            out=mg_t[:, :ndve, :D],
                            in0=mg_t[:, :ndve, :D],
                            in1=mg_t[:, :ndve, D:],
                        )
                    if ndve < HT:
                        # two Pool ops aligned with the sigmoid halves so
                        # each starts as soon as its gates half is ready
                        pool_mul(
                            mg_t[:, ndve:HT, :D],
                            mg_t[:, ndve:HT, :D],
                            mg_t[:, ndve:HT, D:],
                        )
                    if ndve < TILES_PER_CHUNK:
                        pool_mul(
                            mg_t[:, max(ndve, HT) :, :D],
                            mg_t[:, max(ndve, HT) :, :D],
                            mg_t[:, max(ndve, HT) :, D:],
                        )

                    pending_mm2.append((mk, mg_t, c, 0))
                    pending_mm2.append((mk, mg_t, c, 1))
                    c += 1

            for pm in pending_mm2:
                emit_mm2(*pm)
            pending_mm2 = []

            res = out_pool.tile([B, 2 * D], F32)
            nc.vector.tensor_copy(out=res[:], in_=pool12[:])
            nc.sync.dma_start(out_d, res[:])

    return nc


_CACHE: dict = {}


def _get_bass() -> bass.Bass:
    if "nc" not in _CACHE:
        _CACHE["nc"] = build_bass()
    return _CACHE["nc"]


def _prepare_in_maps(nodes, owner_masks, Wt, bt, Wg, bg):
    nodes_h = np.asarray(nodes, dtype=NP_F16)
    masks = np.asarray(owner_masks)
    wt_h = np.asarray(Wt, dtype=NP_F16)
    wg_h = np.asarray(Wg, dtype=NP_F16)
    bt16 = np.asarray(bt, dtype=NP_F16)
    bg16 = np.asarray(bg, dtype=NP_F16)

    c16 = np.zeros((P, CW16), dtype=NP_F16)
    c16[:, 0:1024] = np.tile(bt16[None, :], (P, CHUNK // D))
    c16[:, 1024:1152] = wt_h
    c16[:, 1152:1280] = wg_h
    c16[0, 1280:2304] = np.tile(bg16, CHUNK // D)
    c16[0, 3328:3456] = 1.0
    c16[:, 2304:3328] = np.tile(bg16[None, :], (P, CHUNK // D))

    in_maps = []
    for core in range(N_CORES):
        off = core * N_PER_CORE
        ncr = np.zeros((P, N_PAD), dtype=NP_F16)
        ncr[:, :N_PER_CORE] = nodes_h[off : off + N_PER_CORE].T
        mp = np.zeros((B, N_PAD), dtype=NP_F16)
        mp[:, :N_PER_CORE] = masks[:, off : off + N_PER_CORE]
        mkt = np.ascontiguousarray(mp.reshape(B, N_TILES, P).transpose(2, 1, 0))
        in_maps.append(
            {
                "nodesT": ncr,
                "masksT": mkt,
                "c16": c16,
            }
        )
    return in_maps


def run(inputs: dict, trace: bool = False):
    """Run the kernel. Returns (pooled [B, D] float32, BassKernelResults)."""
    nc = _get_bass()
    in_maps = _prepare_in_maps(**inputs)
    rb = run_bass_kernel_spmd(
        nc, in_maps, core_ids=list(range(N_CORES)), trace=trace
    )
    parts = np.stack([r["out"].astype(np.float64) for r in rb.results])
    tot = parts.sum(axis=0)
    bt64 = np.asarray(inputs["bt"], dtype=np.float64)
    pooled = tot[:, :D] + tot[:, D:] * bt64[None, :]
    return pooled.astype(np.float32), rb


def kernel(**inputs) -> np.ndarray:
    try:
        out, _ = run(inputs, trace=False)
    except Exception:
        # transient device errors (e.g. residual bad state from a previous
        # crashed NEFF) have been observed once; one retry clears them
        out, _ = run(inputs, trace=False)
    return out


if __name__ == "__main__":
    rng = np.random.default_rng(0)
    demo = {
        "nodes": rng.standard_normal((N_TOTAL, S), dtype=np.float32),
        "owner_masks": rng.integers(0, 2, (B, N_TOTAL)).astype(np.int32),
        "Wt": rng.standard_normal((S, D), dtype=np.float32) * 0.09,
        "bt": rng.standard_normal(D).astype(np.float32) * 0.09,
        "Wg": rng.standard_normal((S, D), dtype=np.float32) * 0.09,
        "bg": rng.standard_normal(D).astype(np.float32) * 0.09,
    }
    out = kernel(**demo)
    print(out.shape, out.dtype, np.abs(out).mean())
